# revision 27
# baseline (speedup 1.0000x reference)
"""Trainium2 Bass kernel for the separable transpose-conv (wavelet synthesis) layer.

Full op: x [16, 128, 128, 144] f32 -> out [16, 256, 256, 16] f32.
Two passes of grouped 1D transpose convs (stride 2, 9 taps, 3ch->1ch) with
symmetric padding + border multipliers, separable over W then H.

Formulation: each pass folds (symmetric pad + border multiplier + polyphase
transpose conv + crop) into a constant banded matrix A[cc] of shape [128, 256]
per within-triplet channel cc (columns 0:128 = even outputs, 128:256 = odd).

  pass 1 (W):  z[b,h,g,v]   = sum_w sum_cc x[b,h,w,3g+cc] * A[cc][w,v]
  pass 2 (H):  o[b,m,G2,v]  = sum_h sum_gg A[gg][h,m] * z[b,h,3G2+gg,v]

Both passes map onto PE matmuls with the spatial conv axis as the contraction
(partition) dim; the 3-way channel mixing becomes 3 PSUM-accumulated matmuls.
H == W == 128 so the same A matrices serve both passes.

Sharding: pure data parallel, batch 16 -> 2 per core across 8 cores (SPMD).
"""

import numpy as np

N_CORES = 8
B_FULL = 16
B_PER = B_FULL // N_CORES  # 2
H = 128
W = 128
C = 144
G = C // 3    # 48
G2 = C // 9   # 16

# Matmul dtype: bf16 streams 1 row/cycle with hidden weight loads (measured
# 109ns/MM @ N=256); fp32/fp32r pay a 2-4x weight-load/row penalty. Inputs are
# cast to bf16 host-side, which also halves the HBM load traffic.
_USE_BF16 = True


def _build_A():
    """A [3, 128, 256] f32: banded matrices with pad reflection + border
    multiplier folded in. Validated against the jax reference to ~1e-7 rel."""
    t = np.arange(27, dtype=np.float64).reshape(3, 9)
    inv = (np.cos(t * np.float32(0.7)).astype(np.float32) * 0.5).astype(np.float32)

    L = 128
    P = L + 6
    R = np.zeros((P, L), np.float32)
    R[0, 2] = 2.0
    R[1, 1] = 1.5
    R[2, 0] = 1.25
    for i in range(L):
        R[3 + i, i] = 1.0
    R[P - 3, L - 1] = 1.25
    R[P - 2, L - 2] = 1.5
    R[P - 1, L - 3] = 2.0

    A = np.zeros((3, L, 256), np.float32)
    for cc in range(3):
        Me = np.zeros((P, L), np.float32)
        Mo = np.zeros((P, L), np.float32)
        for v in range(L):
            for j in range(5):
                Me[v + 5 - j, v] += inv[cc, 2 * j]
            for j in range(4):
                Mo[v + 5 - j, v] += inv[cc, 2 * j + 1]
        A[cc, :, 0:128] = R.T @ Me
        A[cc, :, 128:256] = R.T @ Mo
    return A


_CACHE = {}


def _get_nc():
    if "nc" in _CACHE:
        return _CACHE["nc"]

    import concourse.bacc as bacc
    import concourse.tile as tile
    from concourse import mybir

    f32 = mybir.dt.float32
    dt_mm = mybir.dt.bfloat16 if _USE_BF16 else mybir.dt.float32r

    nc = bacc.Bacc("TRN2", target_bir_lowering=False, debug=False, num_devices=N_CORES)
    # Pass 1 is blocked: the spatial w axis is split into 4 blocks of 32; the
    # contraction (w-block + 2-row halo on each side, x 3 within-triplet
    # channels) packs into 108 partitions, so each (g, block) is ONE matmul
    # producing 64 fully-summed columns (32 even + 32 odd outputs) -- no 3-way
    # PSUM accumulation. Host pre-packs x as [b, (wl,cc | halo dup), g, kb, h].
    x_ext = nc.declare_dram_parameter("x", [B_PER, 108, G, 4, H], dt_mm, isOutput=False)
    a1_ext = nc.declare_dram_parameter("amat1", [108, 256], dt_mm, isOutput=False)
    a_ext = nc.declare_dram_parameter("amat", [128, 3 * 256], dt_mm, isOutput=False)
    # out stored bf16 (upcast to f32 host-side): halves store HBM traffic;
    # adds ~0.05% of output scale in rounding error (validated offline).
    # Layout is the raw PE order [b, r, vh, vblk, c, wv]; the host untangles
    # it to [b, h', w', c] so all device-side copies/DMAs stay contiguous.
    o_ext = nc.declare_dram_parameter(
        "out", [B_PER, 2, 128, 8, G2, 32], dt_mm, isOutput=True)

    # g-range split between the two DGE paths, sized to their measured rates
    # (HW ring ~312 GB/s on 108-partition transfers, SW ring ~112 GB/s):
    # HW carries g 0:38 (tiles A+B), SW carries g 38:48 (tile C) in parallel.
    GA, GB = 24, 38

    with tile.TileContext(nc) as tc:
        with tc.tile_pool(name="const", bufs=1) as cpool, \
             tc.tile_pool(name="xpa", bufs=2) as xpoolA, \
             tc.tile_pool(name="xpb", bufs=2) as xpoolB, \
             tc.tile_pool(name="xpc", bufs=2) as xpoolC, \
             tc.tile_pool(name="yp", bufs=1) as ypool, \
             tc.tile_pool(name="st", bufs=4) as spool, \
             tc.tile_pool(name="zp", bufs=4, space="PSUM") as zpool, \
             tc.tile_pool(name="op", bufs=3, space="PSUM") as opool:

            ap1 = cpool.tile([108, 256], dt_mm, tag="amat1")
            nc.sync.dma_start(out=ap1[:], in_=a1_ext[:])
            amat = cpool.tile([128, 3 * 256], dt_mm, tag="amat")
            nc.sync.dma_start(out=amat[:], in_=a_ext[:])
            amat_mm = amat[:]

            # Loads are issued per batch, just in time: issuing everything up
            # front makes the HW DGE spread its engines across every queued
            # transfer, so even the first chunk's semaphore fires near the end
            # of the whole load (13us of PE idle).
            def issue_loads(b, xa, xb, xc):
                bounds_a = [0, 3, 9, 16, 24] if b == 0 else [0, 12, 24]
                bounds_b = [24, 31, 38] if b == 0 else [24, 38]
                bounds_c = [38, 43, 48]
                for g0, g1 in zip(bounds_a, bounds_a[1:]):
                    nc.sync.dma_start(
                        out=xa[:, g0:g1, :, :], in_=x_ext[b, :, g0:g1, :, :])
                for g0, g1 in zip(bounds_b, bounds_b[1:]):
                    nc.sync.dma_start(
                        out=xb[:, g0 - GA:g1 - GA, :, :],
                        in_=x_ext[b, :, g0:g1, :, :])
                for g0, g1 in zip(bounds_c, bounds_c[1:]):
                    nc.gpsimd.dma_start(
                        out=xc[:, g0 - GB:g1 - GB, :, :],
                        in_=x_ext[b, :, g0:g1, :, :])

            xtiles = []
            for b in range(B_PER):
                xa = xpoolA.tile([108, GA, 4, H], dt_mm, tag="xa")
                xb = xpoolB.tile([108, GB - GA, 4, H], dt_mm, tag="xb")
                xc = xpoolC.tile([108, G - GB, 4, H], dt_mm, tag="xc")
                xtiles.append((xa, xb, xc))
            issue_loads(0, *xtiles[0])

            for b in range(B_PER):
                xa, xb, xc = xtiles[b]

                # ---- pass 1: z[h, (kb, ph, vv)] ----
                y_sb = ypool.tile([128, G, 256], dt_mm, tag="y")
                for g in range(G):
                    if g < GA:
                        x_mm = xa[:, g, :, :]
                    elif g < GB:
                        x_mm = xb[:, g - GA, :, :]
                    else:
                        x_mm = xc[:, g - GB, :, :]
                    zp = zpool.tile([128, 256], f32, tag="z")
                    for kb in range(4):
                        nc.tensor.matmul(
                            out=zp[:, 64 * kb:64 * kb + 64],
                            lhsT=x_mm[:, kb, :],
                            rhs=ap1[:, 64 * kb:64 * kb + 64],
                            start=True,
                            stop=True,
                        )
                    if g % 2 == 0:
                        nc.vector.tensor_copy(y_sb[:, g, :], zp[:])
                    else:
                        nc.scalar.copy(y_sb[:, g, :], zp[:])

                # queue the next batch's loads now: they stream during this
                # batch's pass 2 and ahead of its own stores in the SW queue
                if b + 1 < B_PER:
                    issue_loads(b + 1, *xtiles[b + 1])

                # g -> (G2, gg) view for pass-2 rhs slices
                y_mm = y_sb[:].rearrange(
                    "p (gtwo gg) v -> p gg gtwo v", gg=3)

                # ---- pass 2 + store (raw PE-order layout, contiguous writes) ----
                for r in range(2):  # output-row phase: h' = 2*vh + r
                    stage = spool.tile([128, 8, G2 * 32], dt_mm, tag="stage")
                    out_view = o_ext[b, r].rearrange("vh vblk c wv -> vh (vblk c wv)")
                    for vblk in range(8):
                        # y columns are (kb, ph, vv); vblk = par*4 + wb maps
                        # to column block wb*64 + par*32
                        yc = (vblk % 4) * 64 + (vblk // 4) * 32
                        op = opool.tile([128, G2, 32], f32, tag="o2")
                        for gg in range(3):
                            nc.tensor.matmul(
                                out=op[:],
                                lhsT=amat_mm[:, gg * 256 + r * 128: gg * 256 + r * 128 + 128],
                                rhs=y_mm[:, gg, :, yc:yc + 32],
                                start=(gg == 0),
                                stop=(gg == 2),
                            )
                        if vblk % 2 == 0:
                            nc.vector.tensor_copy(stage[:, vblk, :], op[:])
                        else:
                            nc.scalar.copy(stage[:, vblk, :], op[:])
                            # store each 1024-col chunk as soon as its two
                            # vblks are staged; batch-0 stores ride the SW
                            # ring (free after its load share), batch-1 the
                            # scalar HW ring (the HW pipe is free of loads by
                            # then, and the final drain stays on the fast ring)
                            q = vblk // 2
                            seng = nc.gpsimd if b == 0 else nc.scalar
                            seng.dma_start(
                                out=out_view[:, q * 1024:(q + 1) * 1024],
                                in_=stage[:, 2 * q:2 * q + 2, :])

    nc.compile()
    _CACHE["nc"] = nc
    return nc


def _build_A1():
    """Blocked pass-1 matrix [108, 256]: partition = (wl*3+cc | 96+wd*3+cc high
    halo | 102+wd*3+cc low halo), column = kb*64 + ph*32 + vv (old column
    ph*128 + 32*kb + vv)."""
    A = _build_A()
    ap1 = np.zeros((108, 4, 64), np.float32)
    for kb in range(4):
        cols = np.r_[32 * kb:32 * kb + 32, 128 + 32 * kb:128 + 32 * kb + 32]
        for cc in range(3):
            for wl in range(32):
                ap1[wl * 3 + cc, kb, :] = A[cc, 32 * kb + wl, cols]
            for wd in range(2):
                w = 32 * (kb + 1) + wd
                if w < 128:
                    ap1[96 + wd * 3 + cc, kb, :] = A[cc, w, cols]
                w = 32 * kb - 2 + wd
                if w >= 0:
                    ap1[102 + wd * 3 + cc, kb, :] = A[cc, w, cols]
    return ap1.reshape(108, 256)


def _prep_in_maps(x: np.ndarray):
    import ml_dtypes
    dt_np = ml_dtypes.bfloat16 if _USE_BF16 else np.float32
    # pack x -> [b, (wl,cc)+halo dup = 108, g, kb, h]
    xt = x.transpose(0, 2, 3, 1).astype(dt_np)          # [b, w, c, h]
    xb = xt.reshape(B_FULL, 4, 32, G, 3, H)             # [b, kb, wl, g, cc, h]
    main = xb.transpose(0, 2, 4, 3, 1, 5)               # [b, wl, cc, g, kb, h]
    main = main.reshape(B_FULL, 96, G, 4, H)
    halo = np.zeros((B_FULL, 12, G, 4, H), dt_np)
    hi = xt[:, [32, 33, 64, 65, 96, 97]].reshape(B_FULL, 3, 2, G, 3, H)
    halo[:, :6, :, :3] = hi.transpose(0, 2, 4, 3, 1, 5).reshape(B_FULL, 6, G, 3, H)
    lo = xt[:, [30, 31, 62, 63, 94, 95]].reshape(B_FULL, 3, 2, G, 3, H)
    halo[:, 6:, :, 1:] = lo.transpose(0, 2, 4, 3, 1, 5).reshape(B_FULL, 6, G, 3, H)
    xp = np.ascontiguousarray(np.concatenate([main, halo], axis=1))
    amat1 = np.ascontiguousarray(_build_A1().astype(dt_np))
    amat = np.ascontiguousarray(
        _build_A().transpose(1, 0, 2).reshape(128, 3 * 256).astype(dt_np))
    return [
        {"x": xp[i * B_PER:(i + 1) * B_PER], "amat1": amat1, "amat": amat}
        for i in range(N_CORES)
    ]


def kernel(x: np.ndarray) -> np.ndarray:
    from concourse.bass_utils import run_bass_kernel_spmd

    assert x.shape == (B_FULL, H, W, C), x.shape
    nc = _get_nc()
    res = run_bass_kernel_spmd(nc, _prep_in_maps(x), list(range(N_CORES)))
    out = np.concatenate(
        [np.asarray(res.results[i]["out"]).astype(np.float32) for i in range(N_CORES)],
        axis=0)
    # device layout [b, r, vh, (par, wb), c, wv] -> [b, 2*vh+r, wb*64+wv*2+par, c]
    out = out.reshape(B_FULL, 2, 128, 2, 4, G2, 32)
    out = out.transpose(0, 2, 1, 4, 6, 3, 5).reshape(B_FULL, 2 * H, 2 * W, G2)
    return np.ascontiguousarray(out)



# revision 28
# speedup vs baseline: 1.0228x; 1.0228x over previous
"""Trainium2 Bass kernel for the separable transpose-conv (wavelet synthesis) layer.

Dense pass-1 variant: 128-partition x layout (full 16-DMA-engine rate, no halo
duplication), 3-way PSUM accumulation over cc. Loads split between the HW ring
(c 0:114 on sync) and the SW ring (c 114:144 on gpsimd), which run in parallel.

Full op: x [16, 128, 128, 144] f32 -> out [16, 256, 256, 16] f32.
Sharding: pure data parallel, batch 16 -> 2 per core across 8 cores (SPMD).
"""

import numpy as np

N_CORES = 8
B_FULL = 16
B_PER = B_FULL // N_CORES  # 2
H = 128
W = 128
C = 144
G = C // 3    # 48
G2 = C // 9   # 16
CS = 114      # c split: c<114 (g<38) via HW ring, rest via SW ring

_USE_BF16 = True


def _build_A():
    """A [3, 128, 256] f32: banded matrices with pad reflection + border
    multiplier folded in. Validated against the jax reference to ~1e-7 rel."""
    t = np.arange(27, dtype=np.float64).reshape(3, 9)
    inv = (np.cos(t * np.float32(0.7)).astype(np.float32) * 0.5).astype(np.float32)

    L = 128
    P = L + 6
    R = np.zeros((P, L), np.float32)
    R[0, 2] = 2.0
    R[1, 1] = 1.5
    R[2, 0] = 1.25
    for i in range(L):
        R[3 + i, i] = 1.0
    R[P - 3, L - 1] = 1.25
    R[P - 2, L - 2] = 1.5
    R[P - 1, L - 3] = 2.0

    A = np.zeros((3, L, 256), np.float32)
    for cc in range(3):
        Me = np.zeros((P, L), np.float32)
        Mo = np.zeros((P, L), np.float32)
        for v in range(L):
            for j in range(5):
                Me[v + 5 - j, v] += inv[cc, 2 * j]
            for j in range(4):
                Mo[v + 5 - j, v] += inv[cc, 2 * j + 1]
        A[cc, :, 0:128] = R.T @ Me
        A[cc, :, 128:256] = R.T @ Mo
    return A


_CACHE = {}


def _get_nc():
    if "nc" in _CACHE:
        return _CACHE["nc"]

    import concourse.bacc as bacc
    import concourse.tile as tile
    from concourse import mybir

    f32 = mybir.dt.float32
    dt_mm = mybir.dt.bfloat16 if _USE_BF16 else mybir.dt.float32r

    nc = bacc.Bacc("TRN2", target_bir_lowering=False, debug=False, num_devices=N_CORES)
    x_ext = nc.declare_dram_parameter("x", [B_PER, W, C, H], dt_mm, isOutput=False)
    a_ext = nc.declare_dram_parameter("amat", [128, 3 * 256], dt_mm, isOutput=False)
    # out stored bf16 (upcast host-side) in raw PE order [b, r, vh, vblk, c, wv];
    # host untangles to [b, h', w', c] so device copies/DMAs stay contiguous.
    o_ext = nc.declare_dram_parameter(
        "out", [B_PER, 2, 128, 8, G2, 32], dt_mm, isOutput=True)

    with tile.TileContext(nc) as tc:
        with tc.tile_pool(name="const", bufs=1) as cpool, \
             tc.tile_pool(name="xph", bufs=2) as xpoolH, \
             tc.tile_pool(name="xps", bufs=2) as xpoolS, \
             tc.tile_pool(name="yp", bufs=1) as ypool, \
             tc.tile_pool(name="st", bufs=4) as spool, \
             tc.tile_pool(name="zp", bufs=4, space="PSUM") as zpool, \
             tc.tile_pool(name="op", bufs=3, space="PSUM") as opool:

            amat = cpool.tile([128, 3 * 256], dt_mm, tag="amat")
            nc.sync.dma_start(out=amat[:], in_=a_ext[:])
            amat_mm = amat[:]

            def issue_loads(b, xh, xs):
                bounds_h = [0, 12, 45, 80, 114] if b == 0 else [0, 40, 78, 114]
                bounds_s = [114, 129, 144] if b == 0 else [114, 144]
                for c0, c1 in zip(bounds_h, bounds_h[1:]):
                    nc.sync.dma_start(
                        out=xh[:, c0:c1, :], in_=x_ext[b, :, c0:c1, :])
                for c0, c1 in zip(bounds_s, bounds_s[1:]):
                    nc.gpsimd.dma_start(
                        out=xs[:, c0 - CS:c1 - CS, :], in_=x_ext[b, :, c0:c1, :])

            xtiles = []
            for b in range(B_PER):
                xh = xpoolH.tile([128, CS, H], dt_mm, tag="xh")
                xs = xpoolS.tile([128, C - CS, H], dt_mm, tag="xs")
                xtiles.append((xh, xs))
            issue_loads(0, *xtiles[0])

            for b in range(B_PER):
                xh, xs = xtiles[b]

                # ---- pass 1: z[h, v] with 3-way cc accumulation ----
                y_sb = ypool.tile([128, G, 256], dt_mm, tag="y")
                for g in range(G):
                    zp = zpool.tile([128, 256], f32, tag="z")
                    for cc in range(3):
                        c = 3 * g + cc
                        lhs = xh[:, c, :] if c < CS else xs[:, c - CS, :]
                        nc.tensor.matmul(
                            out=zp[:],
                            lhsT=lhs,
                            rhs=amat_mm[:, cc * 256:(cc + 1) * 256],
                            start=(cc == 0),
                            stop=(cc == 2),
                        )
                    if g % 2 == 0:
                        nc.vector.tensor_copy(y_sb[:, g, :], zp[:])
                    else:
                        nc.scalar.copy(y_sb[:, g, :], zp[:])

                # queue the next batch's loads: they stream during this
                # batch's pass 2, ahead of its stores in the SW queue
                if b + 1 < B_PER:
                    issue_loads(b + 1, *xtiles[b + 1])

                y_mm = y_sb[:].rearrange(
                    "p (gtwo gg) v -> p gg gtwo v", gg=3)

                # ---- pass 2 + store (raw PE-order layout, contiguous writes) ----
                for r in range(2):  # output-row phase: h' = 2*vh + r
                    stage = spool.tile([128, 8, G2 * 32], dt_mm, tag="stage")
                    out_view = o_ext[b, r].rearrange("vh vblk c wv -> vh (vblk c wv)")
                    for vblk in range(8):
                        op = opool.tile([128, G2, 32], f32, tag="o2")
                        for gg in range(3):
                            nc.tensor.matmul(
                                out=op[:],
                                lhsT=amat_mm[:, gg * 256 + r * 128: gg * 256 + r * 128 + 128],
                                rhs=y_mm[:, gg, :, vblk * 32:(vblk + 1) * 32],
                                start=(gg == 0),
                                stop=(gg == 2),
                            )
                        if vblk % 2 == 0:
                            nc.vector.tensor_copy(stage[:, vblk, :], op[:])
                        else:
                            nc.scalar.copy(stage[:, vblk, :], op[:])
                            # store each 1024-col chunk as soon as staged;
                            # batch 0 on the SW ring, batch 1 on scalar HW
                            q = vblk // 2
                            seng = nc.gpsimd if b == 0 else nc.scalar
                            seng.dma_start(
                                out=out_view[:, q * 1024:(q + 1) * 1024],
                                in_=stage[:, 2 * q:2 * q + 2, :])

    nc.compile()
    _CACHE["nc"] = nc
    return nc


def _prep_in_maps(x: np.ndarray):
    import ml_dtypes
    dt_np = ml_dtypes.bfloat16 if _USE_BF16 else np.float32
    # [b, h, w, c] -> [b, w, c, h]: contiguous per-partition DMA runs
    xt = np.ascontiguousarray(x.transpose(0, 2, 3, 1).astype(dt_np))
    amat = np.ascontiguousarray(
        _build_A().transpose(1, 0, 2).reshape(128, 3 * 256).astype(dt_np))
    return [
        {"x": xt[i * B_PER:(i + 1) * B_PER], "amat": amat}
        for i in range(N_CORES)
    ]


def kernel(x: np.ndarray) -> np.ndarray:
    from concourse.bass_utils import run_bass_kernel_spmd

    assert x.shape == (B_FULL, H, W, C), x.shape
    nc = _get_nc()
    res = run_bass_kernel_spmd(nc, _prep_in_maps(x), list(range(N_CORES)))
    out = np.concatenate(
        [np.asarray(res.results[i]["out"]).astype(np.float32) for i in range(N_CORES)],
        axis=0)
    # device layout [b, r, vh, (par, wb), c, wv] -> [b, 2*vh+r, wb*64+wv*2+par, c]
    out = out.reshape(B_FULL, 2, 128, 2, 4, G2, 32)
    out = out.transpose(0, 2, 1, 4, 6, 3, 5).reshape(B_FULL, 2 * H, 2 * W, G2)
    return np.ascontiguousarray(out)
